# revision 12
# baseline (speedup 1.0000x reference)
"""DeformableConv2D Trainium2 Bass kernel.

Problem: x[4,64,64,256] f32, w_offset[3,3,256,27], b_offset[27], filt[256,256,3,3]
  -> out[4,64,64,256] f32  (3x3 deformable conv, DG=1, SAME padding)

Sharding: 8 cores = (batch b = core//2) x (image-row half = core%2).
Each core computes 32 output rows (2048 pixels) of its batch element.

Per-core device pipeline:
  P3  offset conv (PE, bf16): wi_cm [27, 2048] = sum_tap W_off^T @ x_cm(shifted)
  PT  PE-transpose wi -> pixel-major wi_pm [128px, 27] tiles
  P4  DVE/ACT: clamp, floor, bilinear weights w00..w11, sigmoid mask,
      int16 gather indices (pixel-major => per-partition scalars)
  P5  SWDGE dma_gather (transpose=False): per (pxgroup, tap, top/bot) fetch
      corner pixel-pairs [128 px, 4 tl, 512(=2px*256c)] bf16 from padded slab
  P6  stage-1: DVE tensor_scalar in-place scale by (w_corner * mask)
      then PE matmuls lhsT=scaledG, rhs=I128  accumulating 4 corners in PSUM
      = fused per-pixel-scale + transpose + corner-sum -> sampled [128c, px] bf16
  P7  stage-2: PE matmuls lhsT=sampled[c,px], rhs=W[c,f] -> out [128px, 256f]
"""

import os
import sys
import numpy as np
import ml_dtypes

sys.path.insert(0, "/opt/trn_rl_repo")

BF16 = ml_dtypes.bfloat16

B, H, W, C, F, K, KK = 4, 64, 64, 256, 256, 3, 9
PAD = 6
Wp = 76
SLAB_ROWS = 45          # 44 addressable + 1 zero guard row
SLAB_PX = SLAB_ROWS * Wp  # 3420
NPX = 2048
CLAMP = 4.99

_CACHE = {}
LAST_RESULT = None
DEBUG = bool(int(os.environ.get("KERNEL_DEBUG", "0")))


def _build_nc():
    import concourse.bass as bass
    from concourse import bacc, mybir
    import concourse.tile as tile

    dt = mybir.dt
    Alu = mybir.AluOpType
    Act = mybir.ActivationFunctionType

    nc = bacc.Bacc("TRN2", target_bir_lowering=False)

    xpm_d = nc.dram_tensor("xslab_pm", [SLAB_PX, C], dt.bfloat16, kind="ExternalInput")
    xcm_d = nc.dram_tensor("xslab_cm", [128, 2 * SLAB_PX], dt.bfloat16, kind="ExternalInput")
    wmain_d = nc.dram_tensor("wmain", [128, 18 * 256], dt.bfloat16, kind="ExternalInput")
    woff_d = nc.dram_tensor("woff", [128, 2 * 9 * 27], dt.bfloat16, kind="ExternalInput")
    bias_d = nc.dram_tensor("bias", [27, 1], dt.float32, kind="ExternalInput")
    out_d = nc.dram_tensor("out", [NPX, C], dt.float32, kind="ExternalOutput")
    if DEBUG:
        dbg_wicm = nc.dram_tensor("dbg_wicm", [27, NPX], dt.float32, kind="ExternalOutput")
        dbg_wipm = nc.dram_tensor("dbg_wipm", [128, 432], dt.float32, kind="ExternalOutput")
        dbg_w = nc.dram_tensor("dbg_w", [128, 5 * 144], dt.float32, kind="ExternalOutput")
        dbg_idx = nc.dram_tensor("dbg_idx", [128, 288], dt.int16, kind="ExternalOutput")
        dbg_idxw = nc.dram_tensor("dbg_idxw", [128, 2304], dt.int16, kind="ExternalOutput")
        dbg_samp = nc.dram_tensor("dbg_samp", [128, 18 * 512], dt.bfloat16, kind="ExternalOutput")
        dbg_g = nc.dram_tensor("dbg_g", [128, 18 * 2048], dt.bfloat16, kind="ExternalOutput")

    # --- inline constants ---
    base_np = np.zeros((128, 144), dtype=np.float32)
    r = np.arange(128)
    for t in range(16):
        for kk in range(KK):
            ki, kj = kk // 3, kk % 3
            base_np[:, t * 9 + kk] = (2 * t + r // 64 + ki + 5) * Wp + (r % 64) + kj + 5
    base_d = nc.inline_tensor(base_np, name="base_tab")
    identb_d = nc.inline_tensor(np.eye(128, dtype=BF16), name="ident_bf")
    identf_d = nc.inline_tensor(np.eye(128, dtype=np.float32), name="ident_f32")

    with tile.TileContext(nc) as tc:
        with tc.tile_pool(name="const", bufs=1) as cpool:
            xcm_sb = cpool.tile([128, 2 * SLAB_PX], dt.bfloat16)
            wmain_sb = cpool.tile([128, 18 * 256], dt.bfloat16)
            woff_sb = cpool.tile([128, 2 * 9 * 27], dt.bfloat16)
            bias_sb = cpool.tile([27, 1], dt.float32)
            base_sb = cpool.tile([128, 144], dt.float32)
            identb_sb = cpool.tile([128, 128], dt.bfloat16)
            identf_sb = cpool.tile([128, 128], dt.float32)
            nc.sync.dma_start(xcm_sb[:], xcm_d[:])
            nc.sync.dma_start(wmain_sb[:], wmain_d[:])
            nc.sync.dma_start(woff_sb[:], woff_d[:])
            nc.sync.dma_start(bias_sb[:], bias_d[:])
            nc.sync.dma_start(base_sb[:], base_d[:])
            nc.sync.dma_start(identb_sb[:], identb_d[:])
            nc.sync.dma_start(identf_sb[:], identf_d[:])

            with tc.tile_pool(name="wi", bufs=1) as wipool:
                wi_cm = wipool.tile([27, NPX], dt.float32)
                wi_pm = wipool.tile([128, 16 * 27], dt.float32)

                # ---------------- P3: offset conv ----------------
                with tc.tile_pool(name="psA", bufs=2, space="PSUM") as psA:
                    for nt in range(4):
                        ps = psA.tile([27, 512], dt.float32, tag="psA")
                        hh = nt * 8
                        first = True
                        for tap in range(9):
                            ki, kj = tap // 3, tap % 3
                            for ch in range(2):
                                lhsT = woff_sb[:, ch * 243 + tap * 27:
                                               ch * 243 + (tap + 1) * 27]
                                off = ch * SLAB_PX + (hh + ki + 5) * Wp + kj + 5
                                rhs = bass.AP(
                                    xcm_sb.tensor, xcm_sb.offset + off,
                                    [list(xcm_sb.ap[0]), [Wp, 8], [1, 64]])
                                nc.tensor.matmul(
                                    ps[:], lhsT, rhs,
                                    start=first, stop=(tap == 8 and ch == 1))
                                first = False
                        # bias + move to sbuf (f32)
                        nc.scalar.activation(
                            wi_cm[:, nt * 512:(nt + 1) * 512], ps[:],
                            Act.Identity, bias=bias_sb[:, 0:1], scale=1.0)

                if DEBUG:
                    nc.sync.dma_start(dbg_wicm[:], wi_cm[:])

                # ---------------- PT: transpose wi to pixel-major ----------------
                with tc.tile_pool(name="psB", bufs=3, space="PSUM") as psB:
                    for t in range(16):
                        pst = psB.tile([128, 27], dt.float32, tag="psB")
                        nc.tensor.transpose(
                            pst[:], wi_cm[:, t * 128:(t + 1) * 128],
                            identf_sb[0:27, 0:27])
                        nc.scalar.copy(wi_pm[:, t * 27:(t + 1) * 27], pst[:])

                # ---------------- P4: weights + indices ----------------
                with tc.tile_pool(name="p4", bufs=1) as p4:
                    o1c = p4.tile([128, 144], dt.float32)
                    o2c = p4.tile([128, 144], dt.float32)
                    fo1 = p4.tile([128, 144], dt.float32)
                    fo2 = p4.tile([128, 144], dt.float32)
                    dy = p4.tile([128, 144], dt.float32)
                    dx = p4.tile([128, 144], dt.float32)
                    dy1 = p4.tile([128, 144], dt.float32)
                    dx1 = p4.tile([128, 144], dt.float32)
                    msig = p4.tile([128, 144], dt.float32)
                    w00 = p4.tile([128, 144], dt.float32)
                    w01 = p4.tile([128, 144], dt.float32)
                    w10 = p4.tile([128, 144], dt.float32)
                    w11 = p4.tile([128, 144], dt.float32)
                    ti32 = p4.tile([128, 144], dt.int32)
                    tf32 = p4.tile([128, 144], dt.float32)
                    gcmp = p4.tile([128, 144], dt.float32)
                    idxf = p4.tile([128, 144], dt.float32)
                    idx16 = p4.tile([128, 288], dt.int16)
                    idxw = p4.tile([128, 2304], dt.int16)

                    def wi_view(ch0):
                        # [128, (t:16), (kk:9)] view of wi_pm at channel block ch0
                        return bass.AP(wi_pm.tensor, wi_pm.offset + ch0,
                                       [list(wi_pm.ap[0]), [27, 16], [1, 9]])

                    v = nc.vector
                    v.tensor_scalar(o1c[:], wi_view(0), CLAMP, -CLAMP, Alu.min, Alu.max)
                    v.tensor_scalar(o2c[:], wi_view(9), CLAMP, -CLAMP, Alu.min, Alu.max)
                    nc.scalar.activation(msig[:], wi_view(18), Act.Sigmoid)
                    # floor(o1c) robust to cast rounding mode
                    v.tensor_copy(ti32[:], o1c[:])
                    v.tensor_copy(tf32[:], ti32[:])
                    v.tensor_tensor(gcmp[:], tf32[:], o1c[:], Alu.is_gt)
                    v.tensor_sub(fo1[:], tf32[:], gcmp[:])
                    v.tensor_copy(ti32[:], o2c[:])
                    v.tensor_copy(tf32[:], ti32[:])
                    v.tensor_tensor(gcmp[:], tf32[:], o2c[:], Alu.is_gt)
                    v.tensor_sub(fo2[:], tf32[:], gcmp[:])
                    v.tensor_sub(dy[:], o1c[:], fo1[:])
                    v.tensor_sub(dx[:], o2c[:], fo2[:])
                    v.tensor_scalar(dy1[:], dy[:], -1.0, 1.0, Alu.mult, Alu.add)
                    v.tensor_scalar(dx1[:], dx[:], -1.0, 1.0, Alu.mult, Alu.add)
                    v.tensor_mul(w00[:], dy1[:], dx1[:])
                    v.tensor_mul(w01[:], dy1[:], dx[:])
                    v.tensor_mul(w10[:], dy[:], dx1[:])
                    v.tensor_mul(w11[:], dy[:], dx[:])
                    # indices
                    v.tensor_scalar_mul(idxf[:], fo1[:], float(Wp))
                    v.tensor_add(idxf[:], idxf[:], fo2[:])
                    v.tensor_add(idxf[:], idxf[:], base_sb[:])
                    # cast to int16 in call-major column order:
                    # idx16 col = (g*72) + kk*8 + tb*4 + tl,
                    # i.e. 4 cols per call-block id (g*9+kk)*2+tb.
                    # src idxf col = t*9+kk with t = g*4+tl -> iterate (g,tl,kk).
                    pi16 = idx16.ap[0][0]
                    for tb in range(2):
                        dst = bass.AP(idx16.tensor, idx16.offset + tb * 4,
                                      [[pi16, 128], [72, 4], [1, 4], [8, 9]])
                        v.tensor_copy(dst, bass.AP(
                            idxf.tensor, idxf.offset,
                            [[idxf.ap[0][0], 128], [36, 4], [9, 4], [1, 9]]))
                        if tb == 0:
                            v.tensor_scalar_add(idxf[:], idxf[:], float(Wp))

                    # wrap indices into dma_gather layout:
                    # call (g, kk, tb) -> idxw cols [blk*32, blk*32+32),
                    # blk = (g*9+kk)*2+tb; wrapped slot of local px l=tl*128+r
                    # is (partition r%16, col tl*8 + r//16).
                    # strip a: src partitions [16a,16a+16) -> dst [0,16)
                    for a in range(8):
                        src = bass.AP(idx16.tensor, idx16.offset + 16 * a * pi16,
                                      [[pi16, 16], [1, 288]])
                        dst = bass.AP(idxw.tensor, idxw.offset + a,
                                      [[idxw.ap[0][0], 16], [32, 72], [8, 4]])
                        nc.sync.dma_start(dst, src)
                    nc.sync.dma_start(idxw[16:32, :], idxw[0:16, :])
                    nc.sync.dma_start(idxw[32:64, :], idxw[0:32, :])
                    nc.sync.dma_start(idxw[64:128, :], idxw[0:64, :])

                    if DEBUG:
                        nc.sync.dma_start(dbg_wipm[:], wi_pm[:])
                        for i, w_ in enumerate((w00, w01, w10, w11, msig)):
                            nc.sync.dma_start(dbg_w[:, i * 144:(i + 1) * 144], w_[:])
                        nc.sync.dma_start(dbg_idx[:], idx16[:])
                        nc.sync.dma_start(dbg_idxw[:], idxw[:])

                    # ---------------- P5/P6/P7 main loop ----------------
                    # Overlapping 2-pixel window view of the slab. Count is
                    # SLAB_PX-1 so the AP extent stays within the tensor
                    # (last addressable idx only reaches the zero guard row).
                    gather_src = bass.AP(xpm_d, 0, [[C, SLAB_PX - 1], [1, 512]])
                    with tc.tile_pool(name="G", bufs=6) as gpool, \
                         tc.tile_pool(name="samp", bufs=36) as spool, \
                         tc.tile_pool(name="osb", bufs=4) as opool, \
                         tc.tile_pool(name="psC", bufs=4, space="PSUM") as psC, \
                         tc.tile_pool(name="psD", bufs=4, space="PSUM") as psD:
                        for g in range(4):
                            samp = {}
                            for kk in range(KK):
                                Gt = []
                                for tb in range(2):
                                    gt = gpool.tile([128, 4, 512], dt.bfloat16, tag="G")
                                    blk = ((g * 9 + kk) * 2 + tb) * 32
                                    nc.gpsimd.dma_gather(
                                        out_ap=gt[:],
                                        in_ap=gather_src,
                                        idxs_ap=idxw[:, blk:blk + 32],
                                        num_idxs=512,
                                        num_idxs_reg=512,
                                        elem_size=512,
                                        elem_step=C,
                                    )
                                    if DEBUG and g == 0:
                                        blk = (kk * 2 + tb) * 2048
                                        nc.sync.dma_start(
                                            dbg_g[:, blk:blk + 2048],
                                            bass.AP(gt.tensor, gt.offset,
                                                    [list(gt.ap[0]), [1, 2048]]))
                                    Gt.append(gt)
                                wmat = ((w00, w01), (w10, w11))
                                for tb in range(2):
                                    for tl in range(4):
                                        col = (g * 4 + tl) * 9 + kk
                                        for corner in range(2):
                                            v.tensor_scalar(
                                                Gt[tb][:, tl, corner * 256:(corner + 1) * 256],
                                                Gt[tb][:, tl, corner * 256:(corner + 1) * 256],
                                                wmat[tb][corner][:, col:col + 1],
                                                msig[:, col:col + 1],
                                                Alu.mult, Alu.mult)
                                for ch in range(2):
                                    ps = psC.tile([128, 512], dt.float32, tag="psC")
                                    for tl in range(4):
                                        n = 0
                                        for tb in range(2):
                                            for corner in range(2):
                                                nc.tensor.matmul(
                                                    ps[:, tl * 128:(tl + 1) * 128],
                                                    Gt[tb][:, tl,
                                                           corner * 256 + ch * 128:
                                                           corner * 256 + ch * 128 + 128],
                                                    identb_sb[:],
                                                    start=(n == 0), stop=(n == 3))
                                                n += 1
                                    st = spool.tile([128, 512], dt.bfloat16, tag="samp")
                                    if (kk + ch) % 2 == 0:
                                        v.tensor_copy(st[:], ps[:])
                                    else:
                                        nc.scalar.copy(st[:], ps[:])
                                    samp[(kk, ch)] = st
                                    if DEBUG and g == 0:
                                        blk = (kk * 2 + ch) * 512
                                        nc.sync.dma_start(
                                            dbg_samp[:, blk:blk + 512], st[:])
                            # stage-2
                            for tl in range(4):
                                po = psD.tile([128, 256], dt.float32, tag="psD")
                                n = 0
                                for kk in range(KK):
                                    for ch in range(2):
                                        nc.tensor.matmul(
                                            po[:],
                                            samp[(kk, ch)][:, tl * 128:(tl + 1) * 128],
                                            wmain_sb[:, (kk * 2 + ch) * 256:
                                                     (kk * 2 + ch + 1) * 256],
                                            start=(n == 0), stop=(n == 17))
                                        n += 1
                                ot = opool.tile([128, 256], dt.float32, tag="osb")
                                nc.scalar.copy(ot[:], po[:])
                                row0 = (g * 4 + tl) * 128
                                nc.sync.dma_start(out_d[row0:row0 + 128, :], ot[:])
    nc.finalize()
    return nc


def _pstride(tile_ap):
    """partition step of a tile AP (elements along partition dim pair)."""
    return tile_ap.ap[0][0]


def _host_prep(x, w_offset, b_offset, filt):
    xp = np.zeros((B, 77, Wp, C), dtype=BF16)
    xp[:, PAD:PAD + H, PAD:PAD + W, :] = x.astype(BF16)

    Wm = np.ascontiguousarray(filt.reshape(F, C, KK))
    wmain = np.zeros((128, 18 * 256), dtype=BF16)
    for kk in range(KK):
        for ch in range(2):
            g = kk * 2 + ch
            wmain[:, g * 256:(g + 1) * 256] = Wm[:, ch * 128:(ch + 1) * 128, kk].T.astype(BF16)

    woff = np.zeros((128, 2 * 9 * 27), dtype=BF16)
    for ch in range(2):
        for tap in range(9):
            ki, kj = tap // 3, tap % 3
            woff[:, ch * 243 + tap * 27:ch * 243 + (tap + 1) * 27] = \
                w_offset[ki, kj, ch * 128:(ch + 1) * 128, :].astype(BF16)

    bias = np.ascontiguousarray(b_offset.reshape(27, 1).astype(np.float32))

    in_maps = []
    for core in range(8):
        b, half = core // 2, core % 2
        h0 = 32 * half
        slab = np.ascontiguousarray(xp[b, h0:h0 + SLAB_ROWS].reshape(SLAB_PX, C))
        cm = np.empty((128, 2 * SLAB_PX), dtype=BF16)
        cm[:, 0:SLAB_PX] = slab[:, 0:128].T
        cm[:, SLAB_PX:] = slab[:, 128:256].T
        in_maps.append({
            "xslab_pm": slab,
            "xslab_cm": np.ascontiguousarray(cm),
            "wmain": wmain,
            "woff": woff,
            "bias": bias,
        })
    return in_maps


def kernel(x, w_offset, b_offset, filt):
    global LAST_RESULT
    x = np.asarray(x, dtype=np.float32)
    w_offset = np.asarray(w_offset, dtype=np.float32)
    b_offset = np.asarray(b_offset, dtype=np.float32)
    filt = np.asarray(filt, dtype=np.float32)

    if "nc" not in _CACHE:
        _CACHE["nc"] = _build_nc()
    nc = _CACHE["nc"]

    from concourse.bass_utils import run_bass_kernel_spmd

    in_maps = _host_prep(x, w_offset, b_offset, filt)
    res = run_bass_kernel_spmd(nc, in_maps, core_ids=list(range(8)))
    LAST_RESULT = res

    out = np.zeros((B, H, W, F), dtype=np.float32)
    for core in range(8):
        b, half = core // 2, core % 2
        out[b, 32 * half:32 * half + 32] = res.results[core]["out"].reshape(32, 64, F)
    return out


# revision 13
# speedup vs baseline: 1.6148x; 1.6148x over previous
"""DeformableConv2D Trainium2 Bass kernel.

Problem: x[4,64,64,256] f32, w_offset[3,3,256,27], b_offset[27], filt[256,256,3,3]
  -> out[4,64,64,256] f32  (3x3 deformable conv, DG=1, SAME padding)

Sharding: 8 cores = (batch b = core//2) x (image-row half = core%2).
Each core computes 32 output rows (2048 pixels) of its batch element.

Key layout trick: the host stages a zero-padded, *paired-row* bf16 copy of
the image: record r=(y,x) holds channels of pixels (y,x) AND (y+1,x).  One
2KB dma_gather descriptor per (tap, pixel) then fetches all 4 bilinear
corners [v00|v10|v01|v11] at once, pixel-major.

Per-core device pipeline:
  P3  offset conv (PE, bf16): wi_cm [27, 2048]
  PT  PE-transpose -> pixel-major wi_pm [128px, 27-per-tile]
  P4  DVE/ACT: clamp, floor, bilinear corner weights (mask-folded),
      int16 gather indices (pixel-major => per-partition scalars)
  P5  SWDGE dma_gather: per (pxgroup, tap) fetch [128px, 4tl, 1024] bf16
  P6  scale by w_corner (DVE/ACT tensor_scalar, per-partition) then PE
      matmuls lhsT=scaledG, rhs=I128 accumulating 4 corners in PSUM
      = fused per-pixel-scale + transpose + corner-sum -> sampled [c, px]
  P7  PE matmuls lhsT=sampled[c,px], rhs=W[c,f] -> out [128px, 256f]
"""

import os
import sys
import numpy as np
import ml_dtypes

sys.path.insert(0, "/opt/trn_rl_repo")

BF16 = ml_dtypes.bfloat16

B, H, W, C, F, K, KK = 4, 64, 64, 256, 256, 3, 9
PAD = 6
Wp = 76
SLAB_ROWS = 45           # 44 addressable + 1 zero guard row
SLAB_PX = SLAB_ROWS * Wp  # 3420
NREC = 44 * Wp            # 3344 addressable paired-row records
NPX = 2048
CLAMP = 4.99

_CACHE = {}
LAST_RESULT = None
DEBUG = bool(int(os.environ.get("KERNEL_DEBUG", "0")))


def _build_nc():
    import concourse.bass as bass
    from concourse import bacc, mybir
    import concourse.tile as tile

    dt = mybir.dt
    Alu = mybir.AluOpType
    Act = mybir.ActivationFunctionType

    nc = bacc.Bacc("TRN2", target_bir_lowering=False)

    xrp_d = nc.dram_tensor("xrp", [NREC + 1, 512], dt.bfloat16, kind="ExternalInput")
    xcm_d = nc.dram_tensor("xslab_cm", [128, 2 * SLAB_PX], dt.bfloat16, kind="ExternalInput")
    wmain_d = nc.dram_tensor("wmain", [128, 18 * 256], dt.bfloat16, kind="ExternalInput")
    woff_d = nc.dram_tensor("woff", [128, 2 * 9 * 27], dt.bfloat16, kind="ExternalInput")
    bias_d = nc.dram_tensor("bias", [27, 1], dt.float32, kind="ExternalInput")
    out_d = nc.dram_tensor("out", [NPX, C], dt.float32, kind="ExternalOutput")
    if DEBUG:
        dbg_wicm = nc.dram_tensor("dbg_wicm", [27, NPX], dt.float32, kind="ExternalOutput")
        dbg_wipm = nc.dram_tensor("dbg_wipm", [128, 432], dt.float32, kind="ExternalOutput")
        dbg_w = nc.dram_tensor("dbg_w", [128, 5 * 144], dt.float32, kind="ExternalOutput")
        dbg_idx = nc.dram_tensor("dbg_idx", [128, 144], dt.int16, kind="ExternalOutput")
        dbg_idxw = nc.dram_tensor("dbg_idxw", [128, 1152], dt.int16, kind="ExternalOutput")
        dbg_samp = nc.dram_tensor("dbg_samp", [128, 18 * 512], dt.bfloat16, kind="ExternalOutput")
        dbg_g = nc.dram_tensor("dbg_g", [128, 9 * 4096], dt.bfloat16, kind="ExternalOutput")

    # --- inline constants ---
    base_np = np.zeros((128, 144), dtype=np.float32)
    r = np.arange(128)
    for t in range(16):
        for kk in range(KK):
            ki, kj = kk // 3, kk % 3
            base_np[:, t * 9 + kk] = (2 * t + r // 64 + ki + 5) * Wp + (r % 64) + kj + 5
    base_d = nc.inline_tensor(base_np, name="base_tab")
    identb_d = nc.inline_tensor(np.eye(128, dtype=BF16), name="ident_bf")
    identf_d = nc.inline_tensor(np.eye(128, dtype=np.float32), name="ident_f32")

    with tile.TileContext(nc) as tc:
        with tc.tile_pool(name="const", bufs=1) as cpool:
            xcm_sb = cpool.tile([128, 2 * SLAB_PX], dt.bfloat16)
            wmain_sb = cpool.tile([128, 18 * 256], dt.bfloat16)
            woff_sb = cpool.tile([128, 2 * 9 * 27], dt.bfloat16)
            bias_sb = cpool.tile([27, 1], dt.float32)
            base_sb = cpool.tile([128, 144], dt.float32)
            identb_sb = cpool.tile([128, 128], dt.bfloat16)
            identf_sb = cpool.tile([128, 128], dt.float32)
            nc.sync.dma_start(xcm_sb[:], xcm_d[:])
            nc.sync.dma_start(woff_sb[:], woff_d[:])
            nc.sync.dma_start(bias_sb[:], bias_d[:])
            nc.sync.dma_start(base_sb[:], base_d[:])
            nc.sync.dma_start(identb_sb[:], identb_d[:])
            nc.sync.dma_start(identf_sb[:], identf_d[:])
            nc.sync.dma_start(wmain_sb[:], wmain_d[:])

            with tc.tile_pool(name="wi", bufs=1) as wipool:
                wi_cm = wipool.tile([27, NPX], dt.float32)
                wi_pm = wipool.tile([128, 16 * 27], dt.float32)

                # ---------------- P3: offset conv ----------------
                with tc.tile_pool(name="psA", bufs=2, space="PSUM") as psA:
                    for nt in range(4):
                        ps = psA.tile([27, 512], dt.float32, tag="psA")
                        hh = nt * 8
                        first = True
                        for tap in range(9):
                            ki, kj = tap // 3, tap % 3
                            for ch in range(2):
                                lhsT = woff_sb[:, ch * 243 + tap * 27:
                                               ch * 243 + (tap + 1) * 27]
                                off = ch * SLAB_PX + (hh + ki + 5) * Wp + kj + 5
                                rhs = bass.AP(
                                    xcm_sb.tensor, xcm_sb.offset + off,
                                    [list(xcm_sb.ap[0]), [Wp, 8], [1, 64]])
                                nc.tensor.matmul(
                                    ps[:], lhsT, rhs,
                                    start=first, stop=(tap == 8 and ch == 1))
                                first = False
                        nc.scalar.activation(
                            wi_cm[:, nt * 512:(nt + 1) * 512], ps[:],
                            Act.Identity, bias=bias_sb[:, 0:1], scale=1.0)

                if DEBUG:
                    nc.sync.dma_start(dbg_wicm[:], wi_cm[:])

                # ---------------- PT: transpose wi to pixel-major ----------------
                with tc.tile_pool(name="psB", bufs=3, space="PSUM") as psB:
                    for t in range(16):
                        pst = psB.tile([128, 27], dt.float32, tag="psB")
                        nc.tensor.transpose(
                            pst[:], wi_cm[:, t * 128:(t + 1) * 128],
                            identf_sb[0:27, 0:27])
                        nc.scalar.copy(wi_pm[:, t * 27:(t + 1) * 27], pst[:])

                # ---------------- P4: weights + indices ----------------
                with tc.tile_pool(name="p4", bufs=1) as p4:
                    o1c = p4.tile([128, 144], dt.float32)
                    o2c = p4.tile([128, 144], dt.float32)
                    fo1 = p4.tile([128, 144], dt.float32)
                    fo2 = p4.tile([128, 144], dt.float32)
                    dy = p4.tile([128, 144], dt.float32)
                    dx = p4.tile([128, 144], dt.float32)
                    dy1 = p4.tile([128, 144], dt.float32)
                    dx1 = p4.tile([128, 144], dt.float32)
                    msig = p4.tile([128, 144], dt.float32)
                    w00 = p4.tile([128, 144], dt.float32)
                    w01 = p4.tile([128, 144], dt.float32)
                    w10 = p4.tile([128, 144], dt.float32)
                    w11 = p4.tile([128, 144], dt.float32)
                    ti32 = p4.tile([128, 144], dt.int32)
                    tf32 = p4.tile([128, 144], dt.float32)
                    gcmp = p4.tile([128, 144], dt.float32)
                    idxf = p4.tile([128, 144], dt.float32)
                    idx16 = p4.tile([128, 144], dt.int16)
                    idxw = p4.tile([128, 1152], dt.int16)

                    def wi_view(ch0):
                        return bass.AP(wi_pm.tensor, wi_pm.offset + ch0,
                                       [list(wi_pm.ap[0]), [27, 16], [1, 9]])

                    v = nc.vector
                    v.tensor_scalar(o1c[:], wi_view(0), CLAMP, -CLAMP, Alu.min, Alu.max)
                    v.tensor_scalar(o2c[:], wi_view(9), CLAMP, -CLAMP, Alu.min, Alu.max)
                    nc.scalar.activation(msig[:], wi_view(18), Act.Sigmoid)
                    # floor(o1c) robust to cast rounding mode
                    v.tensor_copy(ti32[:], o1c[:])
                    v.tensor_copy(tf32[:], ti32[:])
                    v.tensor_tensor(gcmp[:], tf32[:], o1c[:], Alu.is_gt)
                    v.tensor_sub(fo1[:], tf32[:], gcmp[:])
                    v.tensor_copy(ti32[:], o2c[:])
                    v.tensor_copy(tf32[:], ti32[:])
                    v.tensor_tensor(gcmp[:], tf32[:], o2c[:], Alu.is_gt)
                    v.tensor_sub(fo2[:], tf32[:], gcmp[:])
                    v.tensor_sub(dy[:], o1c[:], fo1[:])
                    v.tensor_sub(dx[:], o2c[:], fo2[:])
                    v.tensor_scalar(dy1[:], dy[:], -1.0, 1.0, Alu.mult, Alu.add)
                    v.tensor_scalar(dx1[:], dx[:], -1.0, 1.0, Alu.mult, Alu.add)
                    # mask-folded corner weights
                    v.tensor_mul(w00[:], dy1[:], dx1[:])
                    v.tensor_mul(w01[:], dy1[:], dx[:])
                    v.tensor_mul(w10[:], dy[:], dx1[:])
                    v.tensor_mul(w11[:], dy[:], dx[:])
                    v.tensor_mul(w00[:], w00[:], msig[:])
                    v.tensor_mul(w01[:], w01[:], msig[:])
                    v.tensor_mul(w10[:], w10[:], msig[:])
                    v.tensor_mul(w11[:], w11[:], msig[:])
                    # gather record index (top-left corner; record holds y,y+1)
                    v.tensor_scalar_mul(idxf[:], fo1[:], float(Wp))
                    v.tensor_add(idxf[:], idxf[:], fo2[:])
                    v.tensor_add(idxf[:], idxf[:], base_sb[:])
                    # cast to int16 in call-major order: idx16 col = g*36+kk*4+tl
                    pi16 = idx16.ap[0][0]
                    dst = bass.AP(idx16.tensor, idx16.offset,
                                  [[pi16, 128], [36, 4], [1, 4], [4, 9]])
                    v.tensor_copy(dst, bass.AP(
                        idxf.tensor, idxf.offset,
                        [[idxf.ap[0][0], 128], [36, 4], [9, 4], [1, 9]]))

                    # wrap into dma_gather layout: call (g, kk) -> idxw cols
                    # [blk*32, blk*32+32), blk = g*9+kk; slot of local px
                    # l=tl*128+r is (partition r%16, col tl*8 + r//16).
                    for a in range(8):
                        src = bass.AP(idx16.tensor, idx16.offset + 16 * a * pi16,
                                      [[pi16, 16], [1, 144]])
                        dstw = bass.AP(idxw.tensor, idxw.offset + a,
                                       [[idxw.ap[0][0], 16], [32, 36], [8, 4]])
                        nc.sync.dma_start(dstw, src)
                    nc.sync.dma_start(idxw[16:32, :], idxw[0:16, :])
                    nc.sync.dma_start(idxw[32:64, :], idxw[0:32, :])
                    nc.sync.dma_start(idxw[64:128, :], idxw[0:64, :])

                    if DEBUG:
                        nc.sync.dma_start(dbg_wipm[:], wi_pm[:])
                        for i, w_ in enumerate((w00, w01, w10, w11, msig)):
                            nc.sync.dma_start(dbg_w[:, i * 144:(i + 1) * 144], w_[:])
                        nc.sync.dma_start(dbg_idx[:], idx16[:])
                        nc.sync.dma_start(dbg_idxw[:], idxw[:])

                    # ---------------- P5/P6/P7 main loop ----------------
                    # 2-record (2KB) window view of the paired-row slab.
                    gather_src = bass.AP(xrp_d, 0, [[512, NREC], [1, 1024]])
                    # corner slot order within a gathered elem:
                    wmap = (w00, w10, w01, w11)  # offsets 0,256,512,768
                    with tc.tile_pool(name="G", bufs=4) as gpool, \
                         tc.tile_pool(name="samp", bufs=36) as spool, \
                         tc.tile_pool(name="osb", bufs=4) as opool, \
                         tc.tile_pool(name="psC", bufs=4, space="PSUM") as psC, \
                         tc.tile_pool(name="psD", bufs=4, space="PSUM") as psD:
                        for g in range(4):
                            samp = {}
                            for kk in range(KK):
                                gt = gpool.tile([128, 4, 1024], dt.bfloat16, tag="G")
                                blk = (g * 9 + kk) * 32
                                nc.gpsimd.dma_gather(
                                    out_ap=gt[:],
                                    in_ap=gather_src,
                                    idxs_ap=idxw[:, blk:blk + 32],
                                    num_idxs=512,
                                    num_idxs_reg=512,
                                    elem_size=1024,
                                    elem_step=512,
                                )
                                if DEBUG and g == 0:
                                    nc.sync.dma_start(
                                        dbg_g[:, kk * 4096:(kk + 1) * 4096],
                                        bass.AP(gt.tensor, gt.offset,
                                                [list(gt.ap[0]), [1, 4096]]))
                                for tl in range(4):
                                    col = (g * 4 + tl) * 9 + kk
                                    for c4 in range(4):
                                        sl = gt[:, tl, c4 * 256:(c4 + 1) * 256]
                                        if c4 == 3:
                                            nc.scalar.activation(
                                                sl, sl, Act.Copy,
                                                scale=wmap[c4][:, col:col + 1])
                                        else:
                                            v.tensor_scalar_mul(
                                                sl, sl, wmap[c4][:, col:col + 1])
                                for ch in range(2):
                                    ps = psC.tile([128, 512], dt.float32, tag="psC")
                                    for tl in range(4):
                                        for c4 in range(4):
                                            nc.tensor.matmul(
                                                ps[:, tl * 128:(tl + 1) * 128],
                                                gt[:, tl, c4 * 256 + ch * 128:
                                                   c4 * 256 + ch * 128 + 128],
                                                identb_sb[:],
                                                start=(c4 == 0), stop=(c4 == 3))
                                    st = spool.tile([128, 512], dt.bfloat16, tag="samp")
                                    if (kk + ch) % 2 == 0:
                                        v.tensor_copy(st[:], ps[:])
                                    else:
                                        nc.scalar.copy(st[:], ps[:])
                                    samp[(kk, ch)] = st
                                    if DEBUG and g == 0:
                                        bs = (kk * 2 + ch) * 512
                                        nc.sync.dma_start(
                                            dbg_samp[:, bs:bs + 512], st[:])
                            # stage-2
                            for tl in range(4):
                                po = psD.tile([128, 256], dt.float32, tag="psD")
                                n = 0
                                for kk in range(KK):
                                    for ch in range(2):
                                        nc.tensor.matmul(
                                            po[:],
                                            samp[(kk, ch)][:, tl * 128:(tl + 1) * 128],
                                            wmain_sb[:, (kk * 2 + ch) * 256:
                                                     (kk * 2 + ch + 1) * 256],
                                            start=(n == 0), stop=(n == 17))
                                        n += 1
                                ot = opool.tile([128, 256], dt.float32, tag="osb")
                                nc.scalar.copy(ot[:], po[:])
                                row0 = (g * 4 + tl) * 128
                                nc.sync.dma_start(out_d[row0:row0 + 128, :], ot[:])
    nc.finalize()
    return nc


def _host_prep(x, w_offset, b_offset, filt):
    xp = np.zeros((B, 77, Wp, C), dtype=BF16)
    xp[:, PAD:PAD + H, PAD:PAD + W, :] = x.astype(BF16)

    Wm = np.ascontiguousarray(filt.reshape(F, C, KK))
    wmain = np.zeros((128, 18 * 256), dtype=BF16)
    for kk in range(KK):
        for ch in range(2):
            g = kk * 2 + ch
            wmain[:, g * 256:(g + 1) * 256] = Wm[:, ch * 128:(ch + 1) * 128, kk].T.astype(BF16)

    woff = np.zeros((128, 2 * 9 * 27), dtype=BF16)
    for ch in range(2):
        for tap in range(9):
            ki, kj = tap // 3, tap % 3
            woff[:, ch * 243 + tap * 27:ch * 243 + (tap + 1) * 27] = \
                w_offset[ki, kj, ch * 128:(ch + 1) * 128, :].astype(BF16)

    bias = np.ascontiguousarray(b_offset.reshape(27, 1).astype(np.float32))

    in_maps = []
    for core in range(8):
        b, half = core // 2, core % 2
        h0 = 32 * half
        slab = np.ascontiguousarray(xp[b, h0:h0 + SLAB_ROWS].reshape(SLAB_PX, C))
        # paired-row records: rec r = [slab[r], slab[r+76]]
        xrp = np.zeros((NREC + 1, 512), dtype=BF16)
        xrp[:NREC, 0:256] = slab[:NREC]
        xrp[:NREC, 256:512] = slab[Wp:NREC + Wp]
        cm = np.empty((128, 2 * SLAB_PX), dtype=BF16)
        cm[:, 0:SLAB_PX] = slab[:, 0:128].T
        cm[:, SLAB_PX:] = slab[:, 128:256].T
        in_maps.append({
            "xrp": xrp,
            "xslab_cm": np.ascontiguousarray(cm),
            "wmain": wmain,
            "woff": woff,
            "bias": bias,
        })
    return in_maps


def kernel(x, w_offset, b_offset, filt):
    global LAST_RESULT
    x = np.asarray(x, dtype=np.float32)
    w_offset = np.asarray(w_offset, dtype=np.float32)
    b_offset = np.asarray(b_offset, dtype=np.float32)
    filt = np.asarray(filt, dtype=np.float32)

    if "nc" not in _CACHE:
        _CACHE["nc"] = _build_nc()
    nc = _CACHE["nc"]

    from concourse.bass_utils import run_bass_kernel_spmd

    in_maps = _host_prep(x, w_offset, b_offset, filt)
    res = run_bass_kernel_spmd(nc, in_maps, core_ids=list(range(8)))
    LAST_RESULT = res

    out = np.zeros((B, H, W, F), dtype=np.float32)
    for core in range(8):
        b, half = core // 2, core % 2
        out[b, 32 * half:32 * half + 32] = res.results[core]["out"].reshape(32, 64, F)
    return out
